# revision 40
# baseline (speedup 1.0000x reference)
"""Trainium2 Bass kernel for nn_CluePredictor_gcn (CluePredictor forward).

Data-parallel over batch: 64 samples -> 8 NeuronCores x 8 samples each.
Weights are replicated; per-sample adjacency build + GCN run independently.

Pipeline per sample (all matmuls on PE in float32r fast path unless noted):
  conv1x1 -> 2-layer highway (xT layout [H,L])
  adjacency: onehot rows Oc[e,:] = [e0[e]=i] + [e1[e]=i] (bf16, exact)
             counts = Oc^T Oc (PE, bf16)   [off-diag == directed-edge counts]
             adjI = max(counts > 0, diag(1 + selfloop))   [= adj + I, exact]
  denom   = row-sum(adjI) via ones-vector matmul; invd = 1/denom
  GCN x3 : uT = (h^T)(adjI) ; uTs = uT * invd ; h' = relu(uTs^T W^T [+2b/d])
           computed in both layouts (dual projection, no transposes)
  epilogue: s = cat . conv_w (M=1 matmuls), y = relu(s+cb),
            logits = Y lin_W^T (batched over 8 samples), word-id mask
"""

import sys

if "/opt/trn_rl_repo" not in sys.path:
    sys.path.insert(0, "/opt/trn_rl_repo")

import ml_dtypes
import numpy as np

B, L, EMB, H, NL, E, MAXA = 64, 512, 400, 256, 3, 512, 32
NCORES = 8
BS = B // NCORES            # samples per core
KE = [128, 128, 128, 16]    # EMB contraction chunks
NT = L // 128               # 4 l-tiles
NH = H // 128               # 2 h-tiles

MM_DT = "float32r"          # "float32r" (fast, ~tf32 mult) or "float32" (4x slower)

_cache = {}


def _build(flags):
    import concourse.bass as bass
    import concourse.tile as tile
    from concourse import bacc, mybir

    f32 = mybir.dt.float32
    f16 = mybir.dt.float16
    bf16 = mybir.dt.bfloat16
    i32 = mybir.dt.int32
    fR = getattr(mybir.dt, MM_DT)
    AF = mybir.ActivationFunctionType
    OP = mybir.AluOpType

    def R(ap):
        return ap

    hw_bias, gcn_bias, conv_bias, lin_bias = flags

    nc = bacc.Bacc(
        "TRN2",
        target_bir_lowering=False,
        debug=False,
        enable_asserts=False,
        num_devices=NCORES,
    )

    names = []

    def din(name, shape, dt):
        names.append(name)
        return nc.dram_tensor(name, shape, dt, kind="ExternalInput").ap()

    semb_d = din("semb", [BS, 100, 4, L], fR)       # [s, p, kchunk, t]
    wy_d = din("wy", [BS, L + 2], i32)              # wid | y1 | y2
    edges_d = din("edges", [BS, E, 2], i32)
    w1t_d = din("w1t", [100, 4, H], fR)
    hw2_d = din("hw2", [128, 2, 2, NH, H], fR)      # [p, g/l, layer, kchunk, o]
    gcnwt_d = din("gcnwt", [128, NL, NH, H], fR)
    linwt_d = din("linwt", [L, L], fR)
    iota16_d = din("iota16", [L], f16)
    iota32_d = din("iota32", [MAXA], f32)
    mega_d = din("mega", [128, 263 + NL * H], fR)   # ident|convw|ones|ones_row|conv_w bcast
    i512_d = din("i512", [128, NT, L], bf16)
    if hw_bias:
        hwgb_d = din("hwgb", [128, 4], f32)   # [p, layer*2+half]
        hwlb_d = din("hwlb", [128, 4], f32)
    if gcn_bias:
        gcnb2_d = din("gcnb2", [1, NL * H], fR)   # 2*gcn_b rows
    if conv_bias:
        convb_d = din("convb", [1, 1], f32)
    if lin_bias:
        linb_d = din("linb", [1, L], fR)

    lg_d = nc.dram_tensor("logits", [BS, L], f32, kind="ExternalOutput").ap()
    ma_d = nc.dram_tensor("maska", [BS, MAXA], f32, kind="ExternalOutput").ap()

    def bcast_dram(ap_1d, parts):
        # DMA-side partition broadcast of a 1-D dram row
        return bass.AP(
            tensor=ap_1d.tensor, offset=ap_1d.offset, ap=[[0, parts]] + list(ap_1d.ap)
        )

    with tile.TileContext(nc) as tc:
        from contextlib import ExitStack

        with ExitStack() as ctx:
            singles = ctx.enter_context(tc.tile_pool(name="singles", bufs=1))
            sembp = ctx.enter_context(tc.tile_pool(name="sembp", bufs=2))
            xp = ctx.enter_context(tc.tile_pool(name="xp", bufs=16))
            ocp = ctx.enter_context(tc.tile_pool(name="ocp", bufs=10))
            adjp = ctx.enter_context(tc.tile_pool(name="adjp", bufs=8))
            hnp = ctx.enter_context(tc.tile_pool(name="hnp", bufs=16))
            utp = ctx.enter_context(tc.tile_pool(name="utp", bufs=4))
            htp = ctx.enter_context(tc.tile_pool(name="htp", bufs=4))
            smallp = ctx.enter_context(tc.tile_pool(name="smallp", bufs=8))
            vecp = ctx.enter_context(tc.tile_pool(name="vecp", bufs=2))
            yrowp = ctx.enter_context(tc.tile_pool(name="yrowp", bufs=3))
            ytp = ctx.enter_context(tc.tile_pool(name="ytp", bufs=4))
            dgp = ctx.enter_context(tc.tile_pool(name="dgp", bufs=3))
            miscp = ctx.enter_context(tc.tile_pool(name="miscp", bufs=2))
            psA = ctx.enter_context(tc.tile_pool(name="psA", bufs=2, space="PSUM"))
            ps = ctx.enter_context(tc.tile_pool(name="ps", bufs=4, space="PSUM"))
            pstmp = ctx.enter_context(tc.tile_pool(name="pstmp", bufs=1, space="PSUM"))
            psacc = ctx.enter_context(tc.tile_pool(name="psacc", bufs=1, space="PSUM"))

            # ---------------- constants ----------------
            w1t_sb = singles.tile([100, 4, H], fR, tag="w1t")
            nc.sync.dma_start(out=w1t_sb[:, :, :], in_=w1t_d[:, :, :])


            iota16_sb = singles.tile([128, L], f16, tag="iota16")
            nc.sync.dma_start(out=iota16_sb[:, :], in_=bcast_dram(iota16_d, 128))
            if hw_bias:
                hwgb_sb = singles.tile([128, 4], f32, tag="hwgb")
                nc.sync.dma_start(out=hwgb_sb[:, :], in_=hwgb_d[:, :])
                hwlb_sb = singles.tile([128, 4], f32, tag="hwlb")
                nc.sync.dma_start(out=hwlb_sb[:, :], in_=hwlb_d[:, :])
            if gcn_bias:
                gcnb2_sb = singles.tile([1, NL * H], fR, tag="gcnb2")
                nc.sync.dma_start(out=gcnb2_sb[:, :], in_=gcnb2_d[:, :])
            if conv_bias:
                convb_sb = singles.tile([1, 1], f32, tag="convb")
                nc.sync.dma_start(out=convb_sb[:, :], in_=convb_d[:, :])
            if lin_bias:
                linb_sb = singles.tile([1, L], fR, tag="linb")
                nc.sync.dma_start(out=linb_sb[:, :], in_=linb_d[:, :])


            Y_sb = singles.tile([BS, L], fR, tag="Y")

            # ---------------- per-sample pipeline ----------------
            for s in range(BS):
                # -- input DMAs (one per tensor) --
                semb_sb = sembp.tile([100, 4, L], fR, tag="semb")
                nc.sync.dma_start(out=semb_sb[:, :, :], in_=semb_d[s])
                e01 = smallp.tile([128, NT, 2], i32, tag="e01")
                nc.sync.dma_start(
                    out=e01[:, :, :],
                    in_=edges_d[s].rearrange("(p c) w -> p c w", c=NT),
                )

                if s == 0:
                    # non-critical constants: loaded after sample-0 inputs so
                    # the first conv/highway matmuls start earlier
                    hw2_sb = singles.tile([128, 2, 2, NH, H], fR, tag="hw2")
                    nc.sync.dma_start(out=hw2_sb[:, :, :, :, :], in_=hw2_d[:, :, :, :, :])
                    mega_sb = singles.tile([128, 263 + NL * H], fR, tag="mega")
                    nc.sync.dma_start(out=mega_sb[:, :], in_=mega_d[:, :])
                    ident_sb = mega_sb[:, 0:128]
                    convw_sb = mega_sb[:, 128 : 128 + 2 * NL]
                    ones_col = mega_sb[:, 134:135]
                    ones_row = mega_sb[0:1, 135:263]
                    wbc_sb = mega_sb[:, 263 : 263 + NL * H]
                    i512_sb = singles.tile([128, NT, L], bf16, tag="i512")
                    nc.sync.dma_start(out=i512_sb[:, :, :], in_=i512_d[:, :, :])
                    gcnwt_sb = singles.tile([128, NL, NH, H], fR, tag="gcnwt")
                    nc.sync.dma_start(out=gcnwt_sb[:, :, :, :], in_=gcnwt_d[:, :, :, :])
                    linwt_sb = singles.tile([128, NT, L], fR, tag="linwt")
                    nc.sync.dma_start(
                        out=linwt_sb[:, :, :],
                        in_=linwt_d.rearrange("(k p) m -> p k m", p=128),
                    )
                    iota32_sb = singles.tile([BS, MAXA], f32, tag="iota32")
                    nc.sync.dma_start(out=iota32_sb[:, :], in_=bcast_dram(iota32_d, BS))
                    wy_sb = singles.tile([BS, L + 2], i32, tag="wy")
                    nc.sync.dma_start(out=wy_sb[:, :], in_=wy_d[:, :])
                    wid_sb = wy_sb[:, 0:L]
                    y1_sb = wy_sb[:, L : L + 1]
                    y2_sb = wy_sb[:, L + 1 : L + 2]
                    neg30_sb = singles.tile([BS, L], f32, tag="neg30")
                    nc.vector.memset(neg30_sb[:, :], -1.0e30)

                # -- conv 1x1:  x0T[m] = (W1 @ semb) in [o, t] layout --
                x = []
                for m in range(NH):
                    x_ps = psA.tile([128, L], f32, tag="psA")
                    for ki in range(4):
                        nc.tensor.matmul(
                            out=x_ps[:, :],
                            lhsT=R(w1t_sb[:, ki, m * 128 : (m + 1) * 128]),
                            rhs=R(semb_sb[:, ki, :]),
                            start=(ki == 0),
                            stop=(ki == 3),
                        )
                    xt = xp.tile([128, L], fR, tag="x")
                    nc.scalar.copy(xt[:, :], x_ps[:, :])
                    x.append(xt)

                # -- highway x2 --
                for i in range(2):
                    gs, ds_ = [], []
                    for m in range(NH):
                        g_ps = psA.tile([128, L], f32, tag="psA")
                        l_ps = psA.tile([128, L], f32, tag="psA")
                        for k in range(NH):
                            nc.tensor.matmul(
                                out=g_ps[:, :],
                                lhsT=R(hw2_sb[:, 0, i, k, m * 128 : (m + 1) * 128]),
                                rhs=R(x[k][:, :]),
                                start=(k == 0),
                                stop=(k == NH - 1),
                            )
                        for k in range(NH):
                            nc.tensor.matmul(
                                out=l_ps[:, :],
                                lhsT=R(hw2_sb[:, 1, i, k, m * 128 : (m + 1) * 128]),
                                rhs=R(x[k][:, :]),
                                start=(k == 0),
                                stop=(k == NH - 1),
                            )
                        g = xp.tile([128, L], fR, tag="x")
                        if hw_bias:
                            nc.scalar.activation(
                                g[:, :], g_ps[:, :], AF.Sigmoid,
                                bias=hwgb_sb[:, 2 * i + m : 2 * i + m + 1],
                            )
                        else:
                            nc.scalar.activation(g[:, :], g_ps[:, :], AF.Sigmoid)
                        d = xp.tile([128, L], fR, tag="x")
                        if hw_bias:
                            nc.vector.scalar_tensor_tensor(
                                out=d[:, :], in0=l_ps[:, :],
                                scalar=hwlb_sb[:, 2 * i + m : 2 * i + m + 1],
                                in1=x[m][:, :], op0=OP.add, op1=OP.subtract,
                            )
                        else:
                            nc.vector.tensor_tensor(
                                out=d[:, :], in0=l_ps[:, :], in1=x[m][:, :],
                                op=OP.subtract,
                            )
                        gs.append(g)
                        ds_.append(d)
                    xn = []
                    for m in range(NH):
                        p = xp.tile([128, L], fR, tag="x")
                        nc.gpsimd.tensor_tensor(
                            out=p[:, :], in0=gs[m][:, :], in1=ds_[m][:, :], op=OP.mult
                        )
                        x2 = xp.tile([128, L], fR, tag="x")
                        nc.gpsimd.tensor_tensor(
                            out=x2[:, :], in0=p[:, :], in1=x[m][:, :], op=OP.add
                        )
                        xn.append(x2)
                    x = xn

                # -- adjacency: onehots, counts, threshold --
                e0h = smallp.tile([128, NT], f32, tag="e0h")
                nc.vector.tensor_copy(e0h[:, :], e01[:, :, 0])
                e1h = smallp.tile([128, NT], f32, tag="e1h")
                nc.vector.tensor_copy(e1h[:, :], e01[:, :, 1])
                mself = smallp.tile([128, NT], bf16, tag="mself")
                oc = []
                for c in range(NT):
                    t0 = ocp.tile([128, L], bf16, tag="oc0")
                    nc.vector.tensor_scalar(
                        out=t0[:, :], in0=iota16_sb[:, :],
                        scalar1=e0h[:, c : c + 1], scalar2=None, op0=OP.is_equal,
                    )
                    occ = ocp.tile([128, L], bf16, tag="oc")
                    nc.vector.scalar_tensor_tensor(
                        out=occ[:, :], in0=iota16_sb[:, :],
                        scalar=e1h[:, c : c + 1], in1=t0[:, :],
                        op0=OP.is_equal, op1=OP.add,
                    )
                    oc.append(occ)
                    nc.vector.tensor_tensor(
                        out=mself[:, c : c + 1], in0=e0h[:, c : c + 1],
                        in1=e1h[:, c : c + 1], op=OP.is_equal,
                    )

                # selfloop fix: sfix[i] = 1 + 0.5 * sum_e mself[e] * Oc[e, i]
                # (Oc row of a self-edge is 2*onehot, so the matmul yields
                #  2*selfloop_count; diag target is 1 + selfloop.)
                sfix = smallp.tile([128, NT], f32, tag="sfix")
                sd_ps = pstmp.tile([128, NT], f32, tag="pss")
                for m in range(NT):
                    for c in range(NT):
                        nc.tensor.matmul(
                            out=sd_ps[:, m : m + 1],
                            lhsT=oc[c][:, m * 128 : (m + 1) * 128],
                            rhs=mself[:, c : c + 1],
                            start=(c == 0),
                            stop=(c == NT - 1),
                        )
                nc.scalar.activation(
                    sfix[:, :], sd_ps[:, :], AF.Copy, bias=1.0, scale=0.5
                )

                adji = []
                for m in range(NT):
                    cnt_ps = ps.tile([128, L], f32, tag="ps")
                    for c in range(NT):
                        nc.tensor.matmul(
                            out=cnt_ps[:, :],
                            lhsT=oc[c][:, m * 128 : (m + 1) * 128],
                            rhs=oc[c][:, :],
                            start=(c == 0),
                            stop=(c == NT - 1),
                        )
                    dg = dgp.tile([128, L], bf16, tag="dg")
                    nc.vector.tensor_scalar(
                        out=dg[:, :], in0=i512_sb[:, m, :],
                        scalar1=sfix[:, m : m + 1], scalar2=None, op0=OP.mult,
                    )
                    a = adjp.tile([128, L], fR, tag="adji")
                    nc.vector.scalar_tensor_tensor(
                        out=a[:, :], in0=cnt_ps[:, :], scalar=0.0,
                        in1=dg[:, :], op0=OP.is_gt, op1=OP.max,
                    )
                    adji.append(a)

                # -- denom / inverse --
                d_ps = pstmp.tile([1, L], f32, tag="pss")
                for k in range(NT):
                    nc.tensor.matmul(
                        out=d_ps[:, :], lhsT=R(ones_col[:, :]), rhs=R(adji[k][:, :]),
                        start=(k == 0), stop=(k == NT - 1),
                    )
                invd_row = miscp.tile([1, L], fR, tag="invd_row")
                with nc.allow_low_precision("f32r tag on full-f32 bits"):
                    nc.vector.reciprocal(invd_row[:, :], d_ps[:, :])
                bc_ps = ps.tile([128, L], f32, tag="ps")
                nc.tensor.matmul(
                    out=bc_ps[:, :], lhsT=R(ones_row[:, :]), rhs=R(invd_row[:, :]),
                    start=True, stop=True,
                )
                invd_bc = miscp.tile([128, L], f32, tag="invd_bc")
                nc.scalar.copy(invd_bc[:, :], bc_ps[:, :])

                # -- transpose x2 -> h0 natural [l, h'] --
                hn = []
                for t in range(NT):
                    h_ps = ps.tile([128, H], fR, tag="ps")
                    for m in range(NH):
                        nc.tensor.transpose(
                            h_ps[:, m * 128 : (m + 1) * 128],
                            x[m][:, t * 128 : (t + 1) * 128],
                            ident_sb[:, :],
                        )
                    hh = hnp.tile([128, H], fR, tag="hn")
                    nc.scalar.copy(hh[:, :], h_ps[:, :])
                    hn.append(hh)

                # -- GCN layers --
                s_ps = psacc.tile([1, L], f32, tag="s_ps")
                for layer in range(NL):
                    uts = []
                    for m in range(NH):
                        u_ps = ps.tile([128, L], f32, tag="ps")
                        for k in range(NT):
                            nc.tensor.matmul(
                                out=u_ps[:, :],
                                lhsT=R(hn[k][:, m * 128 : (m + 1) * 128]),
                                rhs=R(adji[k][:, :]),
                                start=(k == 0),
                                stop=(k == NT - 1),
                            )
                        ut = utp.tile([128, L], fR, tag="uts")
                        nc.vector.tensor_tensor(
                            out=ut[:, :], in0=u_ps[:, :], in1=invd_bc[:, :], op=OP.mult
                        )
                        uts.append(ut)
                    hnext = []
                    for t in range(NT):
                        p_ps = ps.tile([128, H], f32, tag="ps")
                        for k in range(NH):
                            nc.tensor.matmul(
                                out=p_ps[:, :],
                                lhsT=R(uts[k][:, t * 128 : (t + 1) * 128]),
                                rhs=R(gcnwt_sb[:, layer, k, :]),
                                start=(k == 0),
                                stop=(k == NH - 1 and not gcn_bias),
                            )
                        if gcn_bias:
                            nc.tensor.matmul(
                                out=p_ps[:, :],
                                lhsT=R(invd_row[:, t * 128 : (t + 1) * 128]),
                                rhs=R(gcnb2_sb[:, layer * H : (layer + 1) * H]),
                                start=False,
                                stop=True,
                            )
                        hh = hnp.tile([128, H], fR, tag="hn")
                        nc.scalar.activation(hh[:, :], p_ps[:, :], AF.Relu)
                        hnext.append(hh)
                    for m in range(NH):
                        t_ps = ps.tile([128, L], f32, tag="ps")
                        for k in range(NH):
                            nc.tensor.matmul(
                                out=t_ps[:, :],
                                lhsT=R(gcnwt_sb[:, layer, k, m * 128 : (m + 1) * 128]),
                                rhs=R(uts[k][:, :]),
                                start=(k == 0),
                                stop=(k == NH - 1 and not gcn_bias),
                            )
                        if gcn_bias:
                            nc.tensor.matmul(
                                out=t_ps[:, :],
                                lhsT=R(gcnb2_sb[:, layer * H + m * 128 : layer * H + (m + 1) * 128]),
                                rhs=R(invd_row[:, :]),
                                start=False,
                                stop=True,
                            )
                        ht = htp.tile([128, L], fR, tag="ht")
                        nc.scalar.activation(ht[:, :], t_ps[:, :], AF.Relu)
                        nc.tensor.matmul(
                            out=s_ps[:, :],
                            lhsT=R(convw_sb[:, 2 * layer + m : 2 * layer + m + 1]),
                            rhs=R(ht[:, :]),
                            start=(layer == 0 and m == 0),
                            stop=(layer == NL - 1 and m == NH - 1),
                        )
                    hn = hnext

                # -- y row (ACT writes partition-0; DMA places into row s) --
                yrow = yrowp.tile([1, L], fR, tag="yrow")
                if conv_bias:
                    nc.scalar.activation(
                        yrow[:, :], s_ps[:, :], AF.Relu, bias=convb_sb[:, :]
                    )
                else:
                    nc.scalar.activation(yrow[:, :], s_ps[:, :], AF.Relu)
                nc.sync.dma_start(out=Y_sb[s : s + 1, :], in_=yrow[:, :])

            # ---------------- final: logits + masks ----------------
            yts = []
            for c in range(NT):
                tp = ps.tile([128, BS], fR, tag="ps")
                nc.tensor.transpose(
                    tp[:, :], Y_sb[:BS, c * 128 : (c + 1) * 128], ident_sb[:BS, :BS]
                )
                ytc = ytp.tile([128, BS], fR, tag="yt")
                nc.vector.tensor_copy(ytc[:, :], tp[:, :])
                yts.append(ytc)
            lg_ps = ps.tile([BS, L], f32, tag="ps")
            for c in range(NT):
                nc.tensor.matmul(
                    out=lg_ps[:, :], lhsT=R(yts[c][:, :]), rhs=R(linwt_sb[:, c, :]),
                    start=(c == 0), stop=(c == NT - 1 and not lin_bias),
                )
            if lin_bias:
                nc.tensor.matmul(
                    out=lg_ps[:, :], lhsT=R(ones_row[:, :BS]), rhs=R(linb_sb[:, :]),
                    start=False, stop=True,
                )
            msk = vecp.tile([BS, L], i32, tag="msk")
            nc.vector.tensor_scalar(
                out=msk[:, :], in0=wid_sb[:, :], scalar1=0.0, scalar2=None,
                op0=OP.not_equal,
            )
            outlg = vecp.tile([BS, L], f32, tag="outlg")
            nc.vector.select(outlg[:, :], msk[:, :], lg_ps[:, :], neg30_sb[:, :])
            nc.sync.dma_start(out=lg_d[:, :], in_=outlg[:, :])

            y1f = vecp.tile([BS, 1], f32, tag="y1f")
            nc.vector.tensor_copy(y1f[:, :], y1_sb[:, :])
            y2f = vecp.tile([BS, 1], f32, tag="y2f")
            nc.vector.tensor_copy(y2f[:, :], y2_sb[:, :])
            ma = vecp.tile([BS, MAXA], f32, tag="ma")
            nc.vector.scalar_tensor_tensor(
                out=ma[:, :], in0=iota32_sb[:, :], scalar=y1f[:, :],
                in1=y2f[:, :].to_broadcast([BS, MAXA]), op0=OP.add, op1=OP.is_le,
            )
            nc.sync.dma_start(out=ma_d[:, :], in_=ma[:, :])

    nc.compile()
    return nc, names


def _prep_inputs(inputs):
    """Host-side: dtype normalization, weight layout prep, per-core shards."""
    f32 = np.float32
    Semb = np.ascontiguousarray(np.asarray(inputs["Semb"], dtype=f32))
    wid = np.ascontiguousarray(np.asarray(inputs["ans_sent_word_ids"]).astype(np.int32))
    y1 = np.asarray(inputs["y1_in_sent"]).astype(np.int32).reshape(B, 1)
    y2 = np.asarray(inputs["y2_in_sent"]).astype(np.int32).reshape(B, 1)
    edges = np.ascontiguousarray(np.asarray(inputs["edges"]).astype(np.int32))

    W1 = np.asarray(inputs["W1"], dtype=f32)
    hw_lin_W = np.asarray(inputs["hw_lin_W"], dtype=f32)
    hw_lin_b = np.asarray(inputs["hw_lin_b"], dtype=f32)
    hw_gate_W = np.asarray(inputs["hw_gate_W"], dtype=f32)
    hw_gate_b = np.asarray(inputs["hw_gate_b"], dtype=f32)
    gcn_W = np.asarray(inputs["gcn_W"], dtype=f32)
    gcn_b = np.asarray(inputs["gcn_b"], dtype=f32)
    conv_W = np.asarray(inputs["conv_W"], dtype=f32)
    conv_b = np.asarray(inputs["conv_b"], dtype=f32)
    lin_W = np.asarray(inputs["lin_W"], dtype=f32)
    lin_b = np.asarray(inputs["lin_b"], dtype=f32)

    flags = (
        bool(np.any(hw_gate_b) or np.any(hw_lin_b)),
        bool(np.any(gcn_b)),
        bool(np.any(conv_b)),
        bool(np.any(lin_b)),
    )

    consts = {
        "w1t": np.ascontiguousarray(
            W1.T.reshape(4, 100, H).transpose(1, 0, 2)
        ),  # [100, kchunk, H]
        "hw2": np.ascontiguousarray(
            np.stack(
                [
                    hw_gate_W.transpose(0, 2, 1).reshape(2, 2, 128, H).transpose(2, 0, 1, 3),
                    hw_lin_W.transpose(0, 2, 1).reshape(2, 2, 128, H).transpose(2, 0, 1, 3),
                ],
                axis=1,
            )  # [128, g/l, layer, kchunk, H]
        ),
        "gcnwt": np.ascontiguousarray(
            gcn_W.transpose(0, 2, 1).reshape(NL, 2, 128, H).transpose(2, 0, 1, 3)
        ),  # [128, layer, kchunk, H]
        "linwt": np.ascontiguousarray(lin_W.T),
        "iota16": np.arange(L, dtype=np.float16),
        "iota32": np.arange(MAXA, dtype=f32),
    }
    mega = np.zeros((128, 263 + NL * H), dtype=f32)
    mega[:, 0:128] = np.eye(128, dtype=f32)
    mega[:, 128 : 128 + 2 * NL] = conv_W.reshape(2 * NL, 128).T
    mega[:, 134] = 1.0
    mega[0, 135:263] = 1.0
    mega[:, 263:] = np.broadcast_to(conv_W.reshape(1, NL * H), (128, NL * H))
    consts["mega"] = mega
    i512 = np.zeros((128, NT, L), dtype=ml_dtypes.bfloat16)
    for t in range(NT):
        p = np.arange(128)
        i512[p, t, t * 128 + p] = 1.0
    consts["i512"] = i512
    if flags[0]:
        consts["hwgb"] = np.ascontiguousarray(
            hw_gate_b.reshape(2, 2, 128).transpose(2, 0, 1).reshape(128, 4)
        )
        consts["hwlb"] = np.ascontiguousarray(
            hw_lin_b.reshape(2, 2, 128).transpose(2, 0, 1).reshape(128, 4)
        )
    if flags[1]:
        consts["gcnb2"] = np.ascontiguousarray((2.0 * gcn_b).reshape(1, NL * H))
    if flags[2]:
        consts["convb"] = conv_b.reshape(1, 1)
    if flags[3]:
        consts["linb"] = lin_b.reshape(1, L)

    semb4 = np.ascontiguousarray(
        Semb.reshape(B, 4, 100, L).transpose(0, 2, 1, 3)
    )  # [B, 100, kchunk, L]
    wy = np.ascontiguousarray(np.concatenate([wid, y1, y2], axis=1))
    in_maps = []
    for c in range(NCORES):
        sl = slice(c * BS, (c + 1) * BS)
        m = {"semb": semb4[sl], "wy": wy[sl], "edges": edges[sl]}
        m.update(consts)
        in_maps.append(m)
    return flags, in_maps


def _run(inputs, trace=False):
    from concourse import bass_utils

    flags, in_maps = _prep_inputs(inputs)
    key = flags
    if key not in _cache:
        _cache[key] = _build(flags)
    nc, names = _cache[key]
    in_maps = [{k: v for k, v in m.items() if k in names} for m in in_maps]
    res = bass_utils.run_bass_kernel_spmd(
        nc, in_maps, core_ids=list(range(NCORES)), trace=trace
    )
    logits = np.concatenate([r["logits"] for r in res.results], axis=0)
    maska = np.concatenate([r["maska"] for r in res.results], axis=0)
    return (logits, maska), res


def kernel(**inputs):
    (logits, maska), _ = _run(inputs)
    return logits, maska


def sim_time(inputs):
    """Per-core execution time from the local cost-model timeline simulator."""
    flags, _ = _prep_inputs(inputs)
    if flags not in _cache:
        _cache[flags] = _build(flags)
    nc, _ = _cache[flags]
    from concourse.timeline_sim import TimelineSim

    return TimelineSim(nc, trace=False).simulate()


def bench(inputs, iters=8, reps=3):
    """Steady-state per-iteration device time: chain `iters` kernel
    executions inside one jit call (iteration i's outputs become iteration
    i+1's donated output buffers, forcing sequential execution), time the
    whole call, subtract the 1-iter call time to remove dispatch overhead.
    """
    import time

    import jax
    import numpy as np_
    from jax.sharding import Mesh, PartitionSpec
    from jax.experimental.shard_map import shard_map
    from concourse import bass2jax, mybir

    bass2jax.install_neuronx_cc_hook()
    flags, in_maps = _prep_inputs(inputs)
    if flags not in _cache:
        _cache[flags] = _build(flags)
    nc, names = _cache[flags]
    in_maps = [{k: v for k, v in m.items() if k in names} for m in in_maps]

    part_name = nc.partition_id_tensor.name if nc.partition_id_tensor else None
    in_names, out_names, out_avals, zero_outs = [], [], [], []
    for alloc in nc.m.functions[0].allocations:
        if not isinstance(alloc, mybir.MemoryLocationSet):
            continue
        name = alloc.memorylocations[0].name
        if alloc.kind == "ExternalInput":
            if name != part_name:
                in_names.append(name)
        elif alloc.kind == "ExternalOutput":
            out_names.append(name)
            shape = tuple(alloc.tensor_shape)
            dtype = mybir.dt.np(alloc.dtype)
            out_avals.append(jax.core.ShapedArray(shape, dtype))
            zero_outs.append(np_.zeros(shape, dtype))
    n_params = len(in_names)
    all_in_names = tuple(
        in_names + out_names + ([part_name] if part_name else [])
    )

    def _body(*args):
        ins = list(args[:n_params])
        outs = list(args[n_params:])
        extra = [bass2jax.partition_id_tensor()] if part_name else []
        outs = list(
            bass2jax._bass_exec_p.bind(
                *ins,
                *outs,
                *extra,
                out_avals=tuple(out_avals),
                in_names=all_in_names,
                out_names=tuple(out_names),
                lowering_input_output_aliases=(),
                sim_require_finite=True,
                sim_require_nnan=True,
                nc=nc,
            )
        )
        return tuple(outs)

    devices = jax.devices()[:NCORES]
    mesh = Mesh(np_.asarray(devices), ("core",))
    nin = n_params + len(out_names)
    per_core = [[np_.asarray(m[name]) for name in in_names] for m in in_maps]
    concat_in = [
        np_.concatenate([per_core[c][i] for c in range(NCORES)], axis=0)
        for i in range(n_params)
    ]
    concat_zeros = [
        np_.zeros((NCORES * z.shape[0], *z.shape[1:]), z.dtype) for z in zero_outs
    ]

    fk = jax.jit(
        shard_map(
            _body,
            mesh=mesh,
            in_specs=(PartitionSpec("core"),) * nin,
            out_specs=(PartitionSpec("core"),) * len(out_names),
            check_rep=False,
        ),
        keep_unused=True,
    )

    def _tiny(a):
        return a + 1.0

    ftiny = jax.jit(
        shard_map(
            _tiny, mesh=mesh, in_specs=(PartitionSpec("core"),),
            out_specs=PartitionSpec("core"), check_rep=False,
        )
    )

    sh = jax.sharding.NamedSharding(mesh, PartitionSpec("core"))
    dev_in = [jax.device_put(a, sh) for a in concat_in]
    dev_zero = [jax.device_put(a, sh) for a in concat_zeros]
    tiny_in = jax.device_put(np_.zeros((NCORES, 8), np_.float32), sh)

    def timeit(f, args, n):
        r = f(*args)
        jax.block_until_ready(r)
        ts = []
        for _ in range(n):
            t0 = time.perf_counter()
            r = f(*args)
            jax.block_until_ready(r)
            ts.append(time.perf_counter() - t0)
        return min(ts), sorted(ts)[len(ts) // 2]

    floor_best, floor_med = timeit(ftiny, (tiny_in,), reps)
    k_best, k_med = timeit(fk, (*dev_in, *dev_zero), reps)
    per_iter = k_med - floor_med
    times = {"kernel_best": k_best, "kernel_med": k_med,
             "floor_best": floor_best, "floor_med": floor_med}
    return per_iter, times
